# revision 16
# baseline (speedup 1.0000x reference)
"""ExtractSearchWindows Trainium2 kernel (8 NeuronCores, Bass/Tile).

out[b, h, w, dy*cv+dx, ky*8+kx] = uint8(P[b, h+off+dy+ky, w+off+dx+kx])
with P = zero-pad(inputs[:, 0], 7) and off = 3 - search_range.

The output (196.6 MB u8) is a pure byte-replication of a tiny input, so
the kernel is bound by per-core DMA-engine write bandwidth (~425 GB/s
across 16 engines; ~26.6 GB/s/engine for descriptors >= 4 KB, less for
small ones).  Work is sharded over (b, h): each of the 8 cores produces
48 output rows as 384 segments (segment = 40-pixel row chunk) in 3
tiles of 128 partitions.

Device-side expansion: strided uint32 DVE tensor_copies read host-
prepared byte-shifted sub-rows S[seg][v][u][j] (v = dy+ky source row,
u = phi+dx byte shift, j = 4a+4kxp+beta addressing pixel w = 4a+phi,
kx = 4*kxp+beta) and scatter them into out-staging tiles that DMA out
with large contiguous descriptors.

Pipeline fill: a small fast-start slice S0a is DMA'd first so the DVE
starts ~1 us earlier; pixels 0-11 of tile 0 drain via two dy-sliced
blocks (640/960 B descriptors, ~0.73x engine rate -- paid while the
engines would otherwise idle); everything later uses w-chunks with
19.2-32 KB descriptors at full rate, sized so the engines never
starve once the first block lands.
"""
import numpy as np

K = 8
MAX_SR = 3
B, H, W = 2, 192, 320
TP = MAX_SR + K // 2          # 7 pad per side
PW = W + 2 * TP               # 334
NCORES = 8
ROWS_PER_CORE = (B * H) // NCORES   # 48
WSEG = 40
NWSEG = W // WSEG             # 8
NSEG = ROWS_PER_CORE * NWSEG  # 384
NTILE = NSEG // 128           # 3

# sr=2 geometry
CV = 5
OSEG = WSEG * CV * CV * K * K   # 64000 output bytes per segment
PIXB = CV * CV * K * K          # 1600 output bytes per pixel
PIXW = PIXB // 4                # 400 u32 per pixel
DW = CV * K * K // 4            # 80 u32 per (pixel, dy)

NV = 12                       # source rows per segment (CV-1+K)
NU = 8                        # byte shifts u = phi+dx
NJ = 44                       # shifted sub-row bytes
SEGB = NV * NU * NJ           # 4224 S bytes per segment
A_NV, A_NJ = 12, 16           # fast-start slice: all v, j<=15 (a<=1)
A_B = A_NV * NU * A_NJ        # 1536

# persistent SBUF layout (u8 offsets)
S0A_OFF = 0
S_OFF = A_B                   # 1152; S tiles at S_OFF + t*SEGB
PERS_B = S_OFF + NTILE * SEGB  # 13824

import os
SPLIT_QUEUES = os.environ.get("ESW_SPLIT_QUEUES", "0") == "1"

_PROG_CACHE = {}


def _make_host_arrays(x, sr):
    """x: (B,1,H,W) f32 -> per-core dict of host-prepped u8 arrays."""
    off = MAX_SR - sr
    P = np.pad(x[:, 0], ((0, 0), (TP, TP), (TP, TP))).astype(np.uint8)
    cores = []
    st = np.lib.stride_tricks.as_strided
    for c in range(NCORES):
        b = (c * ROWS_PER_CORE) // H
        h0 = (c * ROWS_PER_CORE) % H
        flat = np.ascontiguousarray(P[b]).reshape(-1)
        base = (h0 + off) * PW + off
        # S: all 384 segments fully shifted: (r, s, v, u, j)
        s = st(flat[base:], shape=(ROWS_PER_CORE, NWSEG, NV, NU, NJ),
               strides=(PW, WSEG, PW, 1, 1))
        s = np.ascontiguousarray(s).reshape(NSEG, SEGB)
        # S0a: fast-start slice of tile 0 (v<=8, j<16)
        s0a = st(flat[base:], shape=(16, NWSEG, A_NV, NU, A_NJ),
                 strides=(PW, WSEG, PW, 1, 1))
        s0a = np.ascontiguousarray(s0a).reshape(128, A_B)
        cores.append({"s0a": s0a, "s": s})
    return cores


def _build_program(sr):
    import concourse.bass as bass
    import concourse.bacc as bacc
    import concourse.mybir as mybir
    from concourse import tile

    u8 = mybir.dt.uint8
    u32 = mybir.dt.uint32
    nc = bacc.Bacc("TRN2", debug=False)
    s0a_in = nc.declare_dram_parameter("s0a", [128, A_B], u8, isOutput=False)
    s_in = nc.declare_dram_parameter("s", [NSEG, SEGB], u8, isOutput=False)
    out = nc.declare_dram_parameter("out", [NSEG * OSEG], u8, isOutput=True)

    with tile.TileContext(nc) as tc:
        with tc.tile_pool(name="spool", bufs=1) as sp, \
             tc.tile_pool(name="tpool", bufs=1) as tp:
            PS = sp.tile([128, PERS_B], u8)
            p32 = PS[:].bitcast(u32)
            PP32 = PERS_B // 4

            # host data in, latency-critical first, all on the SP queue
            nc.sync.dma_start(PS[:, S0A_OFF:S0A_OFF + A_B], s0a_in[:, :])
            nc.sync.dma_start(PS[:, S_OFF:S_OFF + SEGB], s_in[0:128, :])
            rest_src = bass.AP(s_in.ap().tensor, 128 * SEGB,
                               [[SEGB, 128], [128 * SEGB, NTILE - 1],
                                [1, SEGB]])
            rest_dst = bass.AP(PS[:].tensor, S_OFF + SEGB,
                               [[PERS_B, 128], [SEGB, NTILE - 1], [1, SEGB]])
            nc.sync.dma_start(rest_dst, rest_src)

            def expand(s_off32, src_st, T, t_pitch32, pix_w32, dys, dy0,
                       a0, an):
                """DVE scatter block: one copy per (dy in dys, phi 0..3).

                Reads S at u32 offset s_off32 (+ dy*sv + phi*su + a*sa),
                writes staging tile T laid out [pixel][dy-dy0][dx][ky][kx]
                with pix_w32 u32 per pixel.
                """
                sv, su, sa = src_st
                t32 = T[:].bitcast(u32)
                for dy in dys:
                    for phi in range(4):
                        src = bass.AP(
                            p32.tensor,
                            s_off32 + dy * sv + phi * su + a0 * sa,
                            [[PP32, 128],
                             [sv, K],           # ky
                             [sa, an],          # a
                             [su, CV],          # dx
                             [1, 2]])           # kx pair
                        dst = bass.AP(
                            t32.tensor,
                            phi * pix_w32 + (dy - dy0) * DW,
                            [[t_pitch32, 128],
                             [2, K],                    # ky
                             [4 * pix_w32, an],         # a
                             [K * K // 4, CV],          # dx
                             [1, 2]])                   # kx pair
                        nc.vector.tensor_copy(dst, src)

            A_ST = (NU * A_NJ // 4, A_NJ // 4, 1)
            S_ST = (NU * NJ // 4, NJ // 4, 1)

            def s_off32(t):
                return (S_OFF + t * SEGB) // 4

            def wchunk(t, a0, an, bufs, tag, split=False):
                """Full-depth w-chunk: pixels 4*a0 .. 4*(a0+an)-1 of tile t."""
                T = tp.tile([128, 20 * PIXB], u8, bufs=bufs, name=tag)
                expand(s_off32(t), S_ST, T, 20 * PIXW, PIXW,
                       (0, 1, 2, 3, 4), 0, a0, an)
                nb = 4 * an * PIXB
                if not split:
                    nc.sync.dma_start(
                        bass.AP(out.ap().tensor,
                                t * 128 * OSEG + 4 * a0 * PIXB,
                                [[OSEG, 128], [1, nb]]),
                        T[0:128, 0:nb])
                else:
                    h = nb // 2
                    for i, eng in enumerate((nc.sync, nc.scalar)):
                        eng.dma_start(
                            bass.AP(out.ap().tensor,
                                    t * 128 * OSEG + 4 * a0 * PIXB + i * h,
                                    [[OSEG, 128], [1, h]]),
                            T[0:128, i * h:(i + 1) * h])

            # ---- tile 0 fill --------------------------------------------
            # g1: dy{0,1} x px 0-7 from the fast-start slice (640 B descs)
            Tg1 = tp.tile([128, 8 * 640], u8, bufs=1)
            expand(S0A_OFF // 4, A_ST, Tg1, 8 * 160, 2 * DW, (0, 1), 0,
                   0, 2)
            nc.sync.dma_start(
                bass.AP(out.ap().tensor, 0,
                        [[OSEG, 128], [PIXB, 8], [1, 640]]),
                Tg1[0:128, 0:5120])
            # g2: dy{2,3,4} x px 0-7, also from the slice (960 B descs)
            Tg2 = tp.tile([128, 8 * 960], u8, bufs=1)
            expand(S0A_OFF // 4, A_ST, Tg2, 8 * 240, 3 * DW, (2, 3, 4), 2,
                   0, 2)
            nc.sync.dma_start(
                bass.AP(out.ap().tensor, 640,
                        [[OSEG, 128], [PIXB, 8], [1, 960]]),
                Tg2[0:128, 0:7680])
            # g3/g4: px 8-23, 24-39 full-depth w-chunks
            wchunk(0, 2, 4, 5, "Tst", split=SPLIT_QUEUES)
            wchunk(0, 6, 4, 5, "Tst", split=SPLIT_QUEUES)

            # ---- steady tiles 1,2: 20px w-chunks ------------------------
            for t in (1, 2):
                for ch in range(2):
                    wchunk(t, 5 * ch, 5, 5, "Tst", split=SPLIT_QUEUES)
    nc.compile()
    return nc


def _numpy_fallback(x, sr):
    cv = 2 * sr + 1
    off = MAX_SR - sr
    P = np.pad(x[:, 0], ((0, 0), (TP, TP), (TP, TP))).astype(np.uint8)
    out = np.empty((B, H, W, cv * cv, K * K), np.uint8)
    for dy in range(cv):
        for dx in range(cv):
            for ky in range(K):
                for kx in range(K):
                    out[:, :, :, dy * cv + dx, ky * K + kx] = \
                        P[:, off + dy + ky:off + dy + ky + H,
                          off + dx + kx:off + dx + kx + W]
    return out


def kernel(inputs, search_range):
    from concourse.bass_utils import run_bass_kernel_spmd

    x = np.asarray(inputs, dtype=np.float32)
    sr = int(np.asarray(search_range))
    if sr != 2 or x.shape != (B, 1, H, W):
        return _numpy_fallback(x, sr)

    if sr not in _PROG_CACHE:
        _PROG_CACHE[sr] = _build_program(sr)
    nc = _PROG_CACHE[sr]

    host = _make_host_arrays(x, sr)
    res = run_bass_kernel_spmd(nc, host, list(range(NCORES)))
    outs = [np.asarray(res.results[c]["out"]) for c in range(NCORES)]
    return np.concatenate(outs).reshape(B, H, W, CV * CV, K * K)
